# revision 31
# baseline (speedup 1.0000x reference)
"""Trainium2 Bass kernel for nn_DynamicKernelSelection (moe_routing).

Strategy
--------
Host (cheap, O(B*C)):
  * Gating in float64 (argmax margins ~4e-4, far above fp32 noise).
  * Samples are grouped by identical (expert1, expert2) signature and the
    groups are cut into chunks of <= 4 samples (PSUM bank = 512 fp32 cols).
    No mixed chunks, so no host fix-up convs are needed.
  * Depthwise conv -> banded Toeplitz lhsT per (channel, kernel column):
    T[h', h] = W[c, dh, dw] at h' = h + (dh-k//2)*dil.  The H-taps become a
    single fp16 128x128 matmul per kernel column; W-shifts are realized by
    accumulating the k_w matmuls into PSUM at shifted column ranges.

Device (SPMD, 8 cores):
  * Channels split 128/8: every core runs 16 channels of EVERY chunk, so
    the instruction stream is uniform across cores (perfect balance).
  * Channel-outer loop: per channel ONE batched DMA each for (all experts'
    Toeplitz weights), (all 16 samples of x), (o1 out), (o2 out).  A
    dma_start costs ~630ns of sequencer issue time, so 4 big DMAs/channel
    (64/core) instead of ~416 keeps the DGE queues off the critical path
    (v2 measured Sync 88% busy / PE 62% busy at 416 DMAs).
  * Everything is single-pass fp16 on the PE (1 cycle/column): tolerance
    is 2e-2 while fp16 single-pass lands ~6e-4.  x, T, o1, o2 all fp16
    (halves DMA); PSUM accumulates in fp32.
  * Per (channel, chunk): k1 matmuls -> PSUM -> evac (DVE) -> o1 slice
    (stage-2 rhs) -> k2 matmuls -> PSUM -> evac (ACT) -> o2 slice.
    Stage-2 is emitted two chunks behind stage-1 so the PE never waits
    on an evac.  Input DMAs issue on SP, output DMAs on ACT (both HWDGE).
"""

import numpy as np

B, C, H, W = 16, 128, 128, 128
N_CORES = 8
CPC = C // N_CORES           # channels per core (16)
DIL1, DIL2 = 1, 3
K1S = {0: 3, 1: 5}           # stage-1 expert -> kernel size
K2S = {0: 7, 1: 9, 2: 11}
NMAX = 4                     # samples per chunk (4*W = 512 = one PSUM bank)

_PROGS = {}                  # signature -> compiled program


# --------------------------------------------------------------- host math
def _gating(x, aw1, ab1, aw2, ab2):
    pooled = x.astype(np.float64).mean(axis=(2, 3))
    l1 = pooled @ aw1.astype(np.float64).T + ab1.astype(np.float64)
    l2 = pooled @ aw2.astype(np.float64).T + ab2.astype(np.float64)
    return l1.argmax(axis=1), l2.argmax(axis=1)


def _band(wk, dil):
    """wk: [C, k, k] fp32 -> banded lhsT stack [C, H, k*H] fp16."""
    k = wk.shape[-1]
    t = np.zeros((C, H, k, H), np.float32)
    tv = t.transpose(1, 3, 0, 2)  # [h', h, C, dw] view
    c0 = k // 2
    for dh in range(k):
        d = (dh - c0) * dil
        h = np.arange(max(0, -d), H - max(0, d))
        tv[h + d, h] = wk[:, dh, :]
    return np.ascontiguousarray(t.reshape(C, H, k * H).astype(np.float16))


def _chunk_samples(idx1, idx2):
    """Group samples by (e1, e2); cut groups into chunks of <= NMAX.
    Returns [(e1, e2, [samples])]."""
    from collections import defaultdict
    groups = defaultdict(list)
    for s in range(B):
        groups[(int(idx1[s]), int(idx2[s]))].append(s)
    chunks = []
    for key in sorted(groups):
        lst = groups[key]
        for i in range(0, len(lst), NMAX):
            chunks.append((key[0], key[1], lst[i:i + NMAX]))
    # big chunks first: the final stage-2 group (kernel tail) is smallest
    chunks.sort(key=lambda c: -len(c[2]))
    return chunks


# ------------------------------------------------------------ device program
def _expert_order(sig):
    """T-block order = first-use order, so early chunks' weights arrive
    first within the per-channel T transfer."""
    k1s, k2s = [], []
    for k1, k2, _ in sig:
        if k1 not in k1s:
            k1s.append(k1)
        if k2 not in k2s:
            k2s.append(k2)
    return k1s, k2s


def _build_program(sig):
    """sig: tuple of (k1, k2, n) per chunk."""
    import concourse.tile as tile
    from concourse import bacc, mybir

    f16 = mybir.dt.float16
    f32 = mybir.dt.float32
    copy_f = mybir.ActivationFunctionType.Copy
    nc = bacc.Bacc("TRN2", target_bir_lowering=False, debug=False,
                   enable_asserts=False, num_devices=N_CORES)

    k1s, k2s = _expert_order(sig)
    # column offset of each expert's Toeplitz block inside the fused T tile
    toff = {}
    tcols = 0
    for k in k1s:
        toff[("s1", k)] = tcols
        tcols += k * H
    for k in k2s:
        toff[("s2", k)] = tcols
        tcols += k * H
    # column offset of each chunk inside the fused x/o1/o2 tiles
    goff = []
    xcols = 0
    for k1, k2, n in sig:
        goff.append(xcols)
        xcols += n * W

    t_d = nc.dram_tensor("t", [CPC, H, tcols], f16, kind="ExternalInput").ap()
    x_d = nc.dram_tensor("x", [CPC, H, xcols], f16, kind="ExternalInput").ap()
    o1_d = nc.dram_tensor("o1", [CPC, H, xcols], f16,
                          kind="ExternalOutput").ap()
    o2_d = nc.dram_tensor("o2", [CPC, H, xcols], f16,
                          kind="ExternalOutput").ap()

    def conv_mms(psum, tt, t0, src, s0, k, dil, n):
        """psum[:, :n*W] += conv(src at col offset s0) with T block at t0."""
        c0 = k // 2
        order = [c0] + [dw for dw in range(k) if dw != c0]
        for j, dw in enumerate(order):
            d = (dw - c0) * dil
            a = max(0, -d)
            ln = W - abs(d)
            nc.tensor.matmul(
                out=psum[:, n * a:n * (a + ln)],
                lhsT=tt[:, t0 + dw * H:t0 + (dw + 1) * H],
                rhs=src[:, s0 + n * (a + d):s0 + n * (a + d + ln)],
                start=(j == 0), stop=(j == len(order) - 1),
                skip_group_check=True)

    with tile.TileContext(nc) as tc:
        with (tc.tile_pool(name="xp", bufs=4) as xp,
              tc.tile_pool(name="o1p", bufs=3) as o1p,
              tc.tile_pool(name="o2p", bufs=3) as o2p,
              tc.tile_pool(name="tp", bufs=3) as tp,
              tc.tile_pool(name="ps", bufs=6, space="PSUM") as ps):
            pend = []
            s2_left = {}          # channel -> stage-2 chunks not yet emitted

            t1cols = sum(k * H for k in k1s)

            def emit_stage2(st):
                g, u, k2, n, o1c, o2c, tt = st
                p2 = ps.tile([128, n * W], f32, tag="ps")
                conv_mms(p2, tt, toff[("s2", k2)], o1c, goff[g], k2, DIL2, n)
                nc.scalar.activation(out=o2c[:, goff[g]:goff[g] + n * W],
                                     in_=p2[:], func=copy_f)
                if u == CPC - 1:
                    # kernel tail: store per-chunk so the last store is small
                    nc.scalar.dma_start(
                        out=o2_d[u][:, goff[g]:goff[g] + n * W],
                        in_=o2c[:, goff[g]:goff[g] + n * W])
                else:
                    s2_left[u] -= 1
                    if s2_left[u] == 0:
                        nc.scalar.dma_start(out=o2_d[u], in_=o2c[:])

            for u in range(CPC):
                tt = tp.tile([128, tcols], f16, tag="t", name=f"tt_{u}")
                if u == 0:
                    # kernel head: stage-1 weights + x land first so the PE
                    # starts ~3us earlier than one fused 1.7MB transfer
                    nc.sync.dma_start(out=tt[:, :t1cols],
                                      in_=t_d[u][:, :t1cols])
                else:
                    nc.sync.dma_start(out=tt[:], in_=t_d[u])
                xc = xp.tile([128, xcols], f16, tag="x", name=f"xc_{u}")
                nc.sync.dma_start(out=xc[:], in_=x_d[u])
                if u == 0:
                    # stage-2 weights load on the (idle) ACT HWDGE queue, in
                    # parallel with x on the SP queue, so the first stage-2
                    # group isn't gated on a serialized 0.9MB transfer
                    nc.scalar.dma_start(out=tt[:, t1cols:],
                                        in_=t_d[u][:, t1cols:])
                o1c = o1p.tile([128, xcols], f16, tag="o1", name=f"o1c_{u}")
                o2c = o2p.tile([128, xcols], f16, tag="o2", name=f"o2c_{u}")
                s2_left[u] = len(sig)
                for g, (k1, k2, n) in enumerate(sig):
                    p1 = ps.tile([128, n * W], f32, tag="ps")
                    conv_mms(p1, tt, toff[("s1", k1)], xc, goff[g],
                             k1, DIL1, n)
                    nc.vector.tensor_copy(
                        out=o1c[:, goff[g]:goff[g] + n * W], in_=p1[:])
                    pend.append((g, u, k2, n, o1c, o2c, tt))
                    if len(pend) > 2:
                        emit_stage2(pend.pop(0))
                    if g == len(sig) - 1:
                        nc.scalar.dma_start(out=o1_d[u], in_=o1c[:])
            while pend:
                emit_stage2(pend.pop(0))
    nc.compile()
    return nc


# ------------------------------------------------------------------- driver
def kernel(x, aw1, ab1, aw2, ab2, w1_3, b1_3, w1_5, b1_5,
           w2_7, b2_7, w2_9, b2_9, w2_11, b2_11):
    from concourse.bass_utils import run_bass_kernel_spmd

    x = np.ascontiguousarray(np.asarray(x, dtype=np.float32))
    assert x.shape == (B, C, H, W)

    idx1, idx2 = _gating(np.asarray(x), np.asarray(aw1), np.asarray(ab1),
                         np.asarray(aw2), np.asarray(ab2))
    chunks = _chunk_samples(idx1, idx2)
    sig = tuple((K1S[e1], K2S[e2], len(ss)) for e1, e2, ss in chunks)

    b1e = [np.asarray(b, np.float32) for b in (b1_3, b1_5)]
    b2e = [np.asarray(b, np.float32) for b in (b2_7, b2_9, b2_11)]

    if sig not in _PROGS:
        _PROGS[sig] = _build_program(sig)
    nc = _PROGS[sig]

    w1e = [np.asarray(w, np.float32)[:, 0] for w in (w1_3, w1_5)]
    w2e = [np.asarray(w, np.float32)[:, 0] for w in (w2_7, w2_9, w2_11)]
    k1s, k2s = _expert_order(sig)
    e1_of = {K1S[e]: e for e in range(2)}
    e2_of = {K2S[e]: e for e in range(3)}
    tparts = [_band(w1e[e1_of[k]], DIL1) for k in k1s]
    tparts += [_band(w2e[e2_of[k]], DIL2) for k in k2s]
    tall = np.concatenate(tparts, axis=2)  # [C, H, tcols] fp16

    # fused interleaved x: per chunk [C, H, W, n] -> concat -> [C, H, xcols]
    xg = []
    for e1, e2, ss in chunks:
        xi = np.stack([x[s] for s in ss], axis=-1)
        xg.append(xi.reshape(C, H, len(ss) * W))
    xall = np.concatenate(xg, axis=2).astype(np.float16)

    in_maps = []
    for core in range(N_CORES):
        cs = slice(core * CPC, (core + 1) * CPC)
        in_maps.append({"t": tall[cs], "x": np.ascontiguousarray(xall[cs])})

    res = run_bass_kernel_spmd(nc, in_maps, list(range(N_CORES)))

    out1 = np.empty((B, C, H, W), np.float32)
    out2 = np.empty((B, C, H, W), np.float32)
    goff = np.cumsum([0] + [len(ss) * W for _, _, ss in chunks])
    for core in range(N_CORES):
        cs = slice(core * CPC, (core + 1) * CPC)
        r = res.results[core]
        o1 = r["o1"].astype(np.float32)
        o2 = r["o2"].astype(np.float32)
        for g, (e1, e2, ss) in enumerate(chunks):
            n = len(ss)
            c1 = o1[:, :, goff[g]:goff[g + 1]].reshape(CPC, H, W, n)
            c2 = o2[:, :, goff[g]:goff[g + 1]].reshape(CPC, H, W, n)
            for j, s in enumerate(ss):
                out1[s, cs] = c1[..., j]
                out2[s, cs] = c2[..., j]

    # host bias add (zero in this problem family; kept for generality).
    if any(b.any() for b in b1e) or any(b.any() for b in b2e):
        corr = {}
        for s in range(B):
            e1, e2 = int(idx1[s]), int(idx2[s])
            b1, b2 = b1e[e1], b2e[e2]
            out1[s] += b1[:, None, None]
            if (e1, e2) not in corr:
                img = np.broadcast_to(b1[:, None, None].astype(np.float64),
                                      (C, H, W))
                corr[(e1, e2)] = _host_conv_nobias(
                    img, w2e[e2].astype(np.float64), DIL2)
            out2[s] += (corr[(e1, e2)] + b2[:, None, None]).astype(np.float32)
    return out1, out2


def _host_conv_nobias(x, wk, dil):
    """x [C,H,W] fp64, wk [C,k,k]: same-padded depthwise conv, no bias."""
    k = wk.shape[-1]
    c0 = k // 2
    out = np.zeros_like(x)
    for dh in range(k):
        for dw in range(k):
            dh_, dw_ = (dh - c0) * dil, (dw - c0) * dil
            hs = slice(max(0, -dh_), H - max(0, dh_))
            ws = slice(max(0, -dw_), W - max(0, dw_))
            hs2 = slice(max(0, dh_), H - max(0, -dh_))
            ws2 = slice(max(0, dw_), W - max(0, -dw_))
            out[:, hs, ws] += wk[:, dh, dw][:, None, None] * x[:, hs2, ws2]
    return out

# revision 32
# speedup vs baseline: 1.0101x; 1.0101x over previous
"""Trainium2 Bass kernel for nn_DynamicKernelSelection (moe_routing).

Strategy
--------
Host (cheap, O(B*C)):
  * Gating in float64 (argmax margins ~4e-4, far above fp32 noise).
  * Samples are grouped by identical (expert1, expert2) signature and the
    groups are cut into chunks of <= 4 samples (PSUM bank = 512 fp32 cols).
    No mixed chunks, so no host fix-up convs are needed.
  * Depthwise conv -> banded Toeplitz lhsT per (channel, kernel column):
    T[h', h] = W[c, dh, dw] at h' = h + (dh-k//2)*dil.  The H-taps become a
    single fp16 128x128 matmul per kernel column; W-shifts are realized by
    accumulating the k_w matmuls into PSUM at shifted column ranges.

Device (SPMD, 8 cores):
  * Channels split 128/8: every core runs 16 channels of EVERY chunk, so
    the instruction stream is uniform across cores (perfect balance).
  * Channel-outer loop: per channel ONE batched DMA each for (all experts'
    Toeplitz weights), (all 16 samples of x), (o1 out), (o2 out).  A
    dma_start costs ~630ns of sequencer issue time, so 4 big DMAs/channel
    (64/core) instead of ~416 keeps the DGE queues off the critical path
    (v2 measured Sync 88% busy / PE 62% busy at 416 DMAs).
  * Everything is single-pass fp16 on the PE (1 cycle/column): tolerance
    is 2e-2 while fp16 single-pass lands ~6e-4.  x, T, o1, o2 all fp16
    (halves DMA); PSUM accumulates in fp32.
  * Per (channel, chunk): k1 matmuls -> PSUM -> evac (DVE) -> o1 slice
    (stage-2 rhs) -> k2 matmuls -> PSUM -> evac (ACT) -> o2 slice.
    Stage-2 is emitted two chunks behind stage-1 so the PE never waits
    on an evac.  Input DMAs issue on SP, output DMAs on ACT (both HWDGE).
"""

import numpy as np

B, C, H, W = 16, 128, 128, 128
N_CORES = 8
CPC = C // N_CORES           # channels per core (16)
DIL1, DIL2 = 1, 3
K1S = {0: 3, 1: 5}           # stage-1 expert -> kernel size
K2S = {0: 7, 1: 9, 2: 11}
NMAX = 4                     # samples per chunk (4*W = 512 = one PSUM bank)

_PROGS = {}                  # signature -> compiled program


# --------------------------------------------------------------- host math
def _gating(x, aw1, ab1, aw2, ab2):
    pooled = x.astype(np.float64).mean(axis=(2, 3))
    l1 = pooled @ aw1.astype(np.float64).T + ab1.astype(np.float64)
    l2 = pooled @ aw2.astype(np.float64).T + ab2.astype(np.float64)
    return l1.argmax(axis=1), l2.argmax(axis=1)


def _band(wk, dil):
    """wk: [C, k, k] fp32 -> banded lhsT stack [C, H, k*H] fp16."""
    k = wk.shape[-1]
    t = np.zeros((C, H, k, H), np.float32)
    tv = t.transpose(1, 3, 0, 2)  # [h', h, C, dw] view
    c0 = k // 2
    for dh in range(k):
        d = (dh - c0) * dil
        h = np.arange(max(0, -d), H - max(0, d))
        tv[h + d, h] = wk[:, dh, :]
    return np.ascontiguousarray(t.reshape(C, H, k * H).astype(np.float16))


def _chunk_samples(idx1, idx2):
    """Group samples by (e1, e2); cut groups into chunks of <= NMAX.
    Returns [(e1, e2, [samples])]."""
    from collections import defaultdict
    groups = defaultdict(list)
    for s in range(B):
        groups[(int(idx1[s]), int(idx2[s]))].append(s)
    chunks = []
    for key in sorted(groups):
        lst = groups[key]
        for i in range(0, len(lst), NMAX):
            chunks.append((key[0], key[1], lst[i:i + NMAX]))
    # big chunks first: the final stage-2 group (kernel tail) is smallest
    chunks.sort(key=lambda c: -len(c[2]))
    return chunks


# ------------------------------------------------------------ device program
def _expert_order(sig):
    """T-block order = first-use order, so early chunks' weights arrive
    first within the per-channel T transfer."""
    k1s, k2s = [], []
    for k1, k2, _ in sig:
        if k1 not in k1s:
            k1s.append(k1)
        if k2 not in k2s:
            k2s.append(k2)
    return k1s, k2s


def _build_program(sig):
    """sig: tuple of (k1, k2, n) per chunk."""
    import concourse.tile as tile
    from concourse import bacc, mybir

    f16 = mybir.dt.float16
    f32 = mybir.dt.float32
    copy_f = mybir.ActivationFunctionType.Copy
    nc = bacc.Bacc("TRN2", target_bir_lowering=False, debug=False,
                   enable_asserts=False, num_devices=N_CORES)

    k1s, k2s = _expert_order(sig)
    # column offset of each expert's Toeplitz block inside the fused T tile
    toff = {}
    tcols = 0
    for k in k1s:
        toff[("s1", k)] = tcols
        tcols += k * H
    for k in k2s:
        toff[("s2", k)] = tcols
        tcols += k * H
    # column offset of each chunk inside the fused x/o1/o2 tiles
    goff = []
    xcols = 0
    for k1, k2, n in sig:
        goff.append(xcols)
        xcols += n * W

    t_d = nc.dram_tensor("t", [CPC, H, tcols], f16, kind="ExternalInput").ap()
    x_d = nc.dram_tensor("x", [CPC, H, xcols], f16, kind="ExternalInput").ap()
    o1_d = nc.dram_tensor("o1", [CPC, H, xcols], f16,
                          kind="ExternalOutput").ap()
    o2_d = nc.dram_tensor("o2", [CPC, H, xcols], f16,
                          kind="ExternalOutput").ap()

    def conv_mms(psum, tt, t0, src, s0, k, dil, n):
        """psum[:, :n*W] += conv(src at col offset s0) with T block at t0."""
        c0 = k // 2
        order = [c0] + [dw for dw in range(k) if dw != c0]
        for j, dw in enumerate(order):
            d = (dw - c0) * dil
            a = max(0, -d)
            ln = W - abs(d)
            nc.tensor.matmul(
                out=psum[:, n * a:n * (a + ln)],
                lhsT=tt[:, t0 + dw * H:t0 + (dw + 1) * H],
                rhs=src[:, s0 + n * (a + d):s0 + n * (a + d + ln)],
                start=(j == 0), stop=(j == len(order) - 1),
                skip_group_check=True)

    with tile.TileContext(nc) as tc:
        with (tc.tile_pool(name="xp", bufs=4) as xp,
              tc.tile_pool(name="o1p", bufs=3) as o1p,
              tc.tile_pool(name="o2p", bufs=3) as o2p,
              tc.tile_pool(name="tp", bufs=3) as tp,
              tc.tile_pool(name="ps", bufs=6, space="PSUM") as ps):
            pend = []
            s2_left = {}          # channel -> stage-2 chunks not yet emitted

            t1cols = sum(k * H for k in k1s)

            def emit_stage2(st):
                g, u, k2, n, o1c, o2c, tt = st
                p2 = ps.tile([128, n * W], f32, tag="ps")
                conv_mms(p2, tt, toff[("s2", k2)], o1c, goff[g], k2, DIL2, n)
                nc.scalar.activation(out=o2c[:, goff[g]:goff[g] + n * W],
                                     in_=p2[:], func=copy_f)
                if u == CPC - 1:
                    # kernel tail: store per-chunk so the last store is small
                    nc.scalar.dma_start(
                        out=o2_d[u][:, goff[g]:goff[g] + n * W],
                        in_=o2c[:, goff[g]:goff[g] + n * W])
                else:
                    s2_left[u] -= 1
                    if s2_left[u] == 0:
                        nc.scalar.dma_start(out=o2_d[u], in_=o2c[:])

            for u in range(CPC):
                tt = tp.tile([128, tcols], f16, tag="t", name=f"tt_{u}")
                if u == 0:
                    # kernel head: stage-1 weights + x land first so the PE
                    # starts ~3us earlier than one fused 1.7MB transfer
                    nc.sync.dma_start(out=tt[:, :t1cols],
                                      in_=t_d[u][:, :t1cols])
                else:
                    nc.sync.dma_start(out=tt[:], in_=t_d[u])
                xc = xp.tile([128, xcols], f16, tag="x", name=f"xc_{u}")
                nc.sync.dma_start(out=xc[:], in_=x_d[u])
                if u == 0:
                    nc.sync.dma_start(out=tt[:, t1cols:],
                                      in_=t_d[u][:, t1cols:])
                o1c = o1p.tile([128, xcols], f16, tag="o1", name=f"o1c_{u}")
                o2c = o2p.tile([128, xcols], f16, tag="o2", name=f"o2c_{u}")
                s2_left[u] = len(sig)
                for g, (k1, k2, n) in enumerate(sig):
                    p1 = ps.tile([128, n * W], f32, tag="ps")
                    conv_mms(p1, tt, toff[("s1", k1)], xc, goff[g],
                             k1, DIL1, n)
                    nc.vector.tensor_copy(
                        out=o1c[:, goff[g]:goff[g] + n * W], in_=p1[:])
                    pend.append((g, u, k2, n, o1c, o2c, tt))
                    if len(pend) > 2:
                        emit_stage2(pend.pop(0))
                    if g == len(sig) - 1:
                        nc.scalar.dma_start(out=o1_d[u], in_=o1c[:])
            while pend:
                emit_stage2(pend.pop(0))
    nc.compile()
    return nc


# ------------------------------------------------------------------- driver
def kernel(x, aw1, ab1, aw2, ab2, w1_3, b1_3, w1_5, b1_5,
           w2_7, b2_7, w2_9, b2_9, w2_11, b2_11):
    from concourse.bass_utils import run_bass_kernel_spmd

    x = np.ascontiguousarray(np.asarray(x, dtype=np.float32))
    assert x.shape == (B, C, H, W)

    idx1, idx2 = _gating(np.asarray(x), np.asarray(aw1), np.asarray(ab1),
                         np.asarray(aw2), np.asarray(ab2))
    chunks = _chunk_samples(idx1, idx2)
    sig = tuple((K1S[e1], K2S[e2], len(ss)) for e1, e2, ss in chunks)

    b1e = [np.asarray(b, np.float32) for b in (b1_3, b1_5)]
    b2e = [np.asarray(b, np.float32) for b in (b2_7, b2_9, b2_11)]

    if sig not in _PROGS:
        _PROGS[sig] = _build_program(sig)
    nc = _PROGS[sig]

    w1e = [np.asarray(w, np.float32)[:, 0] for w in (w1_3, w1_5)]
    w2e = [np.asarray(w, np.float32)[:, 0] for w in (w2_7, w2_9, w2_11)]
    k1s, k2s = _expert_order(sig)
    e1_of = {K1S[e]: e for e in range(2)}
    e2_of = {K2S[e]: e for e in range(3)}
    tparts = [_band(w1e[e1_of[k]], DIL1) for k in k1s]
    tparts += [_band(w2e[e2_of[k]], DIL2) for k in k2s]
    tall = np.concatenate(tparts, axis=2)  # [C, H, tcols] fp16

    # fused interleaved x: per chunk [C, H, W, n] -> concat -> [C, H, xcols]
    xg = []
    for e1, e2, ss in chunks:
        xi = np.stack([x[s] for s in ss], axis=-1)
        xg.append(xi.reshape(C, H, len(ss) * W))
    xall = np.concatenate(xg, axis=2).astype(np.float16)

    in_maps = []
    for core in range(N_CORES):
        cs = slice(core * CPC, (core + 1) * CPC)
        in_maps.append({"t": tall[cs], "x": np.ascontiguousarray(xall[cs])})

    res = run_bass_kernel_spmd(nc, in_maps, list(range(N_CORES)))

    out1 = np.empty((B, C, H, W), np.float32)
    out2 = np.empty((B, C, H, W), np.float32)
    goff = np.cumsum([0] + [len(ss) * W for _, _, ss in chunks])
    for core in range(N_CORES):
        cs = slice(core * CPC, (core + 1) * CPC)
        r = res.results[core]
        o1 = r["o1"].astype(np.float32)
        o2 = r["o2"].astype(np.float32)
        for g, (e1, e2, ss) in enumerate(chunks):
            n = len(ss)
            c1 = o1[:, :, goff[g]:goff[g + 1]].reshape(CPC, H, W, n)
            c2 = o2[:, :, goff[g]:goff[g + 1]].reshape(CPC, H, W, n)
            for j, s in enumerate(ss):
                out1[s, cs] = c1[..., j]
                out2[s, cs] = c2[..., j]

    # host bias add (zero in this problem family; kept for generality).
    if any(b.any() for b in b1e) or any(b.any() for b in b2e):
        corr = {}
        for s in range(B):
            e1, e2 = int(idx1[s]), int(idx2[s])
            b1, b2 = b1e[e1], b2e[e2]
            out1[s] += b1[:, None, None]
            if (e1, e2) not in corr:
                img = np.broadcast_to(b1[:, None, None].astype(np.float64),
                                      (C, H, W))
                corr[(e1, e2)] = _host_conv_nobias(
                    img, w2e[e2].astype(np.float64), DIL2)
            out2[s] += (corr[(e1, e2)] + b2[:, None, None]).astype(np.float32)
    return out1, out2


def _host_conv_nobias(x, wk, dil):
    """x [C,H,W] fp64, wk [C,k,k]: same-padded depthwise conv, no bias."""
    k = wk.shape[-1]
    c0 = k // 2
    out = np.zeros_like(x)
    for dh in range(k):
        for dw in range(k):
            dh_, dw_ = (dh - c0) * dil, (dw - c0) * dil
            hs = slice(max(0, -dh_), H - max(0, dh_))
            ws = slice(max(0, -dw_), W - max(0, dw_))
            hs2 = slice(max(0, dh_), H - max(0, -dh_))
            ws2 = slice(max(0, dw_), W - max(0, -dw_))
            out[:, hs, ws] += wk[:, dh, dw][:, None, None] * x[:, hs2, ws2]
    return out